# revision 7
# baseline (speedup 1.0000x reference)
"""Trainium2 Bass kernel for nn_AttentionModel_35510789603684.

Model: feature proj -> two LSTMs scanning over dim0 (B=8 steps, "batch"=T=2048
tokens) -> banded attention (window 129) -> enhance/mask -> (x*m, w_).

Sharding: the LSTM recurrence couples dim0 (batch), but every T position is an
independent LSTM instance. So we shard T=2048 across 8 cores (256 owned tokens
each) with a 128-token left halo for the attention k-window (recomputed
locally; core 0's halo is zero-padded and masked out). Zero cross-core
communication.

Device layout is feature-major ([features on partitions, tokens free]) so every
matmul contracts over partitions. Attention is computed t-major per 128-token
tile with additive -1e30 band masks, exp WITHOUT max subtraction (scores are
O(10); validated to 1e-5 max-rel vs the fp32 reference), and denominator
+1e-10. The dense (8,2048,2048) w_ output is assembled host-side from the
banded blocks (device only writes 16 MB instead of 128 MB).
"""
import os
import numpy as np
from contextlib import ExitStack

import concourse.bass as bass
import concourse.tile as tile
from concourse import bacc, mybir
from concourse import bass_utils

F32 = mybir.dt.float32
AF = mybir.ActivationFunctionType
NEG = np.float32(-1e30)

B = 8          # batch (LSTM scan steps)
T = 2048       # tokens
FIN = 257      # input features
H = 256        # hidden
NCORES = 8
OWN = T // NCORES          # owned tokens per core = 256
WIN = OWN + 128            # window tokens per core = 384


# ---------------------------------------------------------------- device ----

def build_nc():
    nc = bacc.Bacc("TRN2", target_bir_lowering=False, debug=False,
                   num_devices=NCORES)

    def din(name, shape):
        return nc.dram_tensor(name, shape, F32, kind="ExternalInput").ap()

    def dout(name, shape):
        return nc.dram_tensor(name, shape, F32, kind="ExternalOutput").ap()

    x_t = din("x_t", (B, FIN, WIN))
    featT = din("featT", (FIN, H))
    fb = din("fb", (128, 2))
    kih = din("kih", (H, 4 * H))
    khh = din("khh", (H, 4 * H))
    bk = din("bk", (128, 8))
    qih = din("qih", (H, 4 * H))
    qhh = din("qhh", (H, 4 * H))
    bq = din("bq", (128, 8))
    scoT = din("scoT", (H, H))
    enhT = din("enhT", (2 * H, H))
    eb = din("eb", (128, 2))
    mskT = din("mskT", (H, FIN))
    mb = din("mb", (128, 3))
    mask_std = din("mask_std", (128, 256))
    mask_t0 = din("mask_t0", (128, 256))
    ident = din("ident", (128, 128))

    w_band = dout("w_band", (B, 2, 128, 256))
    y_t = dout("y_t", (B, FIN, OWN))

    with TileProgram(nc) as tp:
        tp.run(x_t, featT, fb, kih, khh, bk, qih, qhh, bq, scoT, enhT, eb,
               mskT, mb, mask_std, mask_t0, ident, w_band, y_t)

    nc.compile()
    return nc


class TileProgram:
    def __init__(self, nc):
        self.nc = nc
        self.ctx = ExitStack()
        self.tc = None

    def __enter__(self):
        self.ctx.__enter__()
        self.tc = self.ctx.enter_context(tile.TileContext(self.nc))
        return self

    def __exit__(self, *a):
        return self.ctx.__exit__(*a)

    def run(self, x_t, featT, fb, kih, khh, bk, qih, qhh, bq, scoT, enhT, eb,
            mskT, mb, mask_std, mask_t0, ident, w_band, y_t):
        nc, tc, ctx = self.nc, self.tc, self.ctx
        pool = lambda name, bufs: ctx.enter_context(
            tc.tile_pool(name=name, bufs=bufs))

        const = pool("const", 1)
        store = pool("store", 1)
        ps = ctx.enter_context(tc.tile_pool(name="ps", bufs=7, space="PSUM"))

        def ldc(ap, shape, name):
            t = const.tile(list(shape), F32, name=name)
            nc.sync.dma_start(out=t, in_=ap)
            return t

        # ---- constants / weights into SBUF -------------------------------
        xb = [[ldc(x_t[b, c * 128:(c + 1) * 128, :], [128, WIN], f"xb{b}_{c}")
               for c in range(2)] for b in range(B)]
        x2 = const.tile([1, B, WIN], F32, name="x2")
        nc.sync.dma_start(out=x2,
                          in_=x_t[:, 256:257, :].rearrange("b o n -> o b n"))

        f_t = [ldc(featT[0:128, :], [128, H], "f0"),
               ldc(featT[128:256, :], [128, H], "f1"),
               ldc(featT[256:257, :], [1, H], "f2")]
        kih_t = [ldc(kih[c * 128:(c + 1) * 128, :], [128, 4 * H], f"kih{c}")
                 for c in range(2)]
        khh_t = [ldc(khh[c * 128:(c + 1) * 128, :], [128, 4 * H], f"khh{c}")
                 for c in range(2)]
        qih_t = [ldc(qih[c * 128:(c + 1) * 128, :], [128, 4 * H], f"qih{c}")
                 for c in range(2)]
        qhh_t = [ldc(qhh[c * 128:(c + 1) * 128, :], [128, 4 * H], f"qhh{c}")
                 for c in range(2)]
        sc_t = [ldc(scoT[c * 128:(c + 1) * 128, :], [128, H], f"sc{c}")
                for c in range(2)]
        en_t = [ldc(enhT[c * 128:(c + 1) * 128, :], [128, H], f"en{c}")
                for c in range(4)]
        mk_t = [ldc(mskT[c * 128:(c + 1) * 128, :], [128, FIN], f"mk{c}")
                for c in range(2)]
        fb_s = ldc(fb, [128, 2], "fb_s")
        bk_s = ldc(bk, [128, 8], "bk_s")
        bq_s = ldc(bq, [128, 8], "bq_s")
        eb_s = ldc(eb, [128, 2], "eb_s")
        mb_s = ldc(mb, [128, 3], "mb_s")
        m_std = ldc(mask_std, [128, 256], "m_std")
        m_t0 = ldc(mask_t0, [128, 256], "m_t0")
        id_s = ldc(ident, [128, 128], "id_s")

        # ---- persistent state / activations ------------------------------
        kfm = [[store.tile([128, WIN], F32, name=f"kfm{b}_{c}")
                for c in range(2)] for b in range(B)]
        qfm = [[store.tile([128, OWN], F32, name=f"qfm{b}_{c}")
                for c in range(2)] for b in range(B)]
        ck = [store.tile([128, WIN], F32, name=f"ck{c}") for c in range(2)]
        cq = [store.tile([128, OWN], F32, name=f"cq{c}") for c in range(2)]

        hp = pool("hp", 3)
        gp = pool("gp", 8)
        tcp = pool("tcp", 3)
        tmp = pool("tmp", 4)
        qsp = pool("qsp", 4)
        ep = pool("ep", 3)
        wp = pool("wp", 3)
        wtp = pool("wtp", 3)
        ktp = pool("ktp", 4)
        smp = pool("smp", 4)
        ctp = pool("ctp", 4)
        op = pool("op", 4)
        mp = pool("mp", 4)
        yp = pool("yp", 4)

        GATE_FUNCS = [AF.Sigmoid, AF.Sigmoid, AF.Tanh, AF.Sigmoid]  # i f g o

        def lstm_gates(b, hcur, ih_t, hh_t, b_s, state, n):
            """Returns list of 8 gate sbuf tiles [128, n] (chunk = gate*2+hc)."""
            outs = []
            for g in range(4):
                for hc in range(2):
                    c = g * 2 + hc
                    gps = ps.tile([128, n], F32, name=f"gps{b}_{c}", tag="ps")
                    nc.tensor.matmul(gps, ih_t[0][:, c * 128:(c + 1) * 128],
                                     hcur[0], start=True, stop=False)
                    nc.tensor.matmul(gps, ih_t[1][:, c * 128:(c + 1) * 128],
                                     hcur[1], start=False, stop=(b == 0))
                    if b > 0:
                        nc.tensor.matmul(gps, hh_t[0][:, c * 128:(c + 1) * 128],
                                         state[0], start=False, stop=False)
                        nc.tensor.matmul(gps, hh_t[1][:, c * 128:(c + 1) * 128],
                                         state[1], start=False, stop=True)
                    gsb = gp.tile([128, n], F32, name=f"g{b}_{c}", tag="gates")
                    nc.scalar.activation(gsb, gps, GATE_FUNCS[g],
                                         bias=b_s[:, c:c + 1])
                    outs.append(gsb)
            return outs

        def lstm_update(b, gates, cstate, hout, n):
            for hc in range(2):
                gi, gf, gg, go = (gates[hc], gates[2 + hc], gates[4 + hc],
                                  gates[6 + hc])
                if b == 0:
                    nc.vector.tensor_mul(cstate[hc], gi, gg)
                else:
                    t1 = tmp.tile([128, n], F32, name=f"t1_{b}_{hc}", tag="tmp")
                    t2 = tmp.tile([128, n], F32, name=f"t2_{b}_{hc}", tag="tmp")
                    nc.vector.tensor_mul(t1, gf, cstate[hc])
                    nc.vector.tensor_mul(t2, gi, gg)
                    nc.vector.tensor_add(cstate[hc], t1, t2)
                tch = tcp.tile([128, n], F32, name=f"tch{b}_{hc}", tag="tch")
                nc.scalar.activation(tch, cstate[hc], AF.Tanh)
                nc.vector.tensor_mul(hout[hc], go, tch)

        for b in range(B):
            # ---- feature projection for batch b: h = tanh(featT.T @ x) ----
            h = []
            for hc in range(2):
                hps = ps.tile([128, WIN], F32, name=f"hps{b}_{hc}", tag="ps")
                nc.tensor.matmul(hps, f_t[0][:, hc * 128:(hc + 1) * 128],
                                 xb[b][0], start=True, stop=False)
                nc.tensor.matmul(hps, f_t[1][:, hc * 128:(hc + 1) * 128],
                                 xb[b][1], start=False, stop=False)
                nc.tensor.matmul(hps, f_t[2][0:1, hc * 128:(hc + 1) * 128],
                                 x2[0:1, b, :], start=False, stop=True)
                ht = hp.tile([128, WIN], F32, name=f"h{b}_{hc}", tag="h")
                nc.scalar.activation(ht, hps, AF.Tanh, bias=fb_s[:, hc:hc + 1])
                h.append(ht)
            hq = [ht[:, 128:WIN] for ht in h]

            # ---- LSTM step b (k over 384 tokens, q over 256 owned) --------
            kg = lstm_gates(b, h, kih_t, khh_t, bk_s,
                            kfm[b - 1] if b else None, WIN)
            lstm_update(b, kg, ck, kfm[b], WIN)
            qg = lstm_gates(b, hq, qih_t, qhh_t, bq_s,
                            qfm[b - 1] if b else None, OWN)
            lstm_update(b, qg, cq, qfm[b], OWN)

            # ---- attention for batch b ------------------------------------
            # qs = score_w @ q   (feature-major)
            qs = []
            for mc in range(2):
                qps = ps.tile([128, OWN], F32, name=f"qps{b}_{mc}", tag="ps")
                nc.tensor.matmul(qps, sc_t[0][:, mc * 128:(mc + 1) * 128],
                                 qfm[b][0], start=True, stop=False)
                nc.tensor.matmul(qps, sc_t[1][:, mc * 128:(mc + 1) * 128],
                                 qfm[b][1], start=False, stop=True)
                qsb = qsp.tile([128, OWN], F32, name=f"qs{b}_{mc}", tag="qs")
                nc.any.tensor_copy(qsb, qps)
                qs.append(qsb)

            # k token-major: ktm[sc] = transpose of k window chunk sc
            ktm = []
            for sc in range(3):
                kps = ps.tile([128, 256], F32, name=f"kps{b}_{sc}", tag="ps")
                for hc in range(2):
                    nc.tensor.transpose(
                        kps[:, hc * 128:(hc + 1) * 128],
                        kfm[b][hc][:, sc * 128:(sc + 1) * 128], id_s)
                ksb = ktp.tile([128, 256], F32, name=f"ktm{b}_{sc}", tag="ktm")
                nc.any.tensor_copy(ksb, kps)
                ktm.append(ksb)

            sums = smp.tile([128, 2], F32, name=f"sums{b}", tag="sums")
            rsum = smp.tile([128, 2], F32, name=f"rsum{b}", tag="rsum")
            wT = []
            for t in range(2):
                sps = ps.tile([128, 256], F32, name=f"sps{b}_{t}", tag="ps")
                nc.tensor.matmul(sps, qs[0][:, t * 128:(t + 1) * 128],
                                 kfm[b][0][:, t * 128:t * 128 + 256],
                                 start=True, stop=False)
                nc.tensor.matmul(sps, qs[1][:, t * 128:(t + 1) * 128],
                                 kfm[b][1][:, t * 128:t * 128 + 256],
                                 start=False, stop=True)
                nc.vector.tensor_add(sps, sps, m_t0 if t == 0 else m_std)
                e = ep.tile([128, 256], F32, name=f"e{b}_{t}", tag="e")
                nc.scalar.activation(e, sps, AF.Exp,
                                     accum_out=sums[:, t:t + 1])
                wT.append(e)
            nc.vector.tensor_scalar_add(rsum, sums, 1e-10)
            nc.vector.reciprocal(rsum, rsum)
            for t in range(2):
                w = wp.tile([128, 256], F32, name=f"w{b}_{t}", tag="w")
                nc.vector.tensor_scalar_mul(w, wT[t], rsum[:, t:t + 1])
                nc.sync.dma_start(out=w_band[b, t], in_=w)
                wps = ps.tile([128, 256], F32, name=f"wps{b}_{t}", tag="ps")
                for sc in range(2):
                    nc.tensor.transpose(wps[:, sc * 128:(sc + 1) * 128],
                                        w[:, sc * 128:(sc + 1) * 128], id_s)
                wtsb = wtp.tile([128, 256], F32, name=f"wt{b}_{t}", tag="wt")
                nc.any.tensor_copy(wtsb, wps)
                wT[t] = wtsb

            cT = []
            for hc in range(2):
                cps = ps.tile([128, OWN], F32, name=f"cps{b}_{hc}", tag="ps")
                for t in range(2):
                    sl = cps[:, t * 128:(t + 1) * 128]
                    nc.tensor.matmul(
                        sl, ktm[t][:, hc * 128:(hc + 1) * 128],
                        wT[t][:, 0:128], start=True, stop=False)
                    nc.tensor.matmul(
                        sl, ktm[t + 1][:, hc * 128:(hc + 1) * 128],
                        wT[t][:, 128:256], start=False, stop=True)
                csb = ctp.tile([128, OWN], F32, name=f"cT{b}_{hc}", tag="cT")
                nc.any.tensor_copy(csb, cps)
                cT.append(csb)

            # ---- enhance -> mask -> y ------------------------------------
            rhs4 = [cT[0], cT[1], qfm[b][0], qfm[b][1]]
            enh = []
            for mc in range(2):
                eps_ = ps.tile([128, OWN], F32, name=f"enps{b}_{mc}", tag="ps")
                for kc in range(4):
                    nc.tensor.matmul(eps_,
                                     en_t[kc][:, mc * 128:(mc + 1) * 128],
                                     rhs4[kc], start=(kc == 0),
                                     stop=(kc == 3))
                osb = op.tile([128, OWN], F32, name=f"enh{b}_{mc}", tag="enh")
                nc.scalar.activation(osb, eps_, AF.Tanh,
                                     bias=eb_s[:, mc:mc + 1])
                enh.append(osb)
            for mc in range(2):
                mps = ps.tile([128, OWN], F32, name=f"mps{b}_{mc}", tag="ps")
                nc.tensor.matmul(mps, mk_t[0][:, mc * 128:(mc + 1) * 128],
                                 enh[0], start=True, stop=False)
                nc.tensor.matmul(mps, mk_t[1][:, mc * 128:(mc + 1) * 128],
                                 enh[1], start=False, stop=True)
                msb = mp.tile([128, OWN], F32, name=f"m{b}_{mc}", tag="m")
                nc.scalar.activation(msb, mps, AF.Sigmoid,
                                     bias=mb_s[:, mc:mc + 1])
                ysb = yp.tile([128, OWN], F32, name=f"y{b}_{mc}", tag="y")
                nc.vector.tensor_mul(ysb, msb, xb[b][mc][:, 128:WIN])
                nc.sync.dma_start(out=y_t[b, mc * 128:(mc + 1) * 128, :],
                                  in_=ysb)
            # feature 256 row
            m2ps = ps.tile([1, OWN], F32, name=f"m2ps{b}", tag="ps")
            nc.tensor.matmul(m2ps, mk_t[0][:, 256:257], enh[0],
                             start=True, stop=False)
            nc.tensor.matmul(m2ps, mk_t[1][:, 256:257], enh[1],
                             start=False, stop=True)
            m2 = mp.tile([1, OWN], F32, name=f"m2_{b}", tag="m2")
            nc.scalar.activation(m2, m2ps, AF.Sigmoid, bias=mb_s[0:1, 2:3])
            y2 = yp.tile([1, OWN], F32, name=f"y2_{b}", tag="y2")
            nc.vector.tensor_mul(y2, m2, x2[0:1, b, 128:WIN])
            nc.sync.dma_start(out=y_t[b, 256:257, :], in_=y2)


# ------------------------------------------------------------------ host ----

_NC_CACHE = {}


def _get_nc():
    if "nc" not in _NC_CACHE:
        _NC_CACHE["nc"] = build_nc()
    return _NC_CACHE["nc"]


def _host_prep(inputs):
    f32 = np.float32
    x = np.asarray(inputs["x"], f32)
    chunk = lambda v: np.ascontiguousarray(
        np.asarray(v, f32).reshape(-1, 128).T)      # (N,) -> (128, N/128)

    i_idx = np.arange(128, dtype=np.int64)[:, None]
    j_idx = np.arange(256, dtype=np.int64)[None, :]
    band = (j_idx >= i_idx) & (j_idx <= i_idx + 128)
    mask_std = np.where(band, f32(0), NEG).astype(f32)
    mask_first = np.where(band & (j_idx >= 128), f32(0), NEG).astype(f32)

    mb3 = np.zeros((128, 3), f32)
    mb_full = np.asarray(inputs["mask_b"], f32)
    mb3[:, 0] = mb_full[0:128]
    mb3[:, 1] = mb_full[128:256]
    mb3[0, 2] = mb_full[256]

    common = {
        "featT": np.ascontiguousarray(np.asarray(inputs["feat_w"], f32).T),
        "fb": chunk(inputs["feat_b"]),
        "kih": np.ascontiguousarray(np.asarray(inputs["k_Wih"], f32).T),
        "khh": np.ascontiguousarray(np.asarray(inputs["k_Whh"], f32).T),
        "bk": chunk(np.asarray(inputs["k_bih"], f32)
                    + np.asarray(inputs["k_bhh"], f32)),
        "qih": np.ascontiguousarray(np.asarray(inputs["q_Wih"], f32).T),
        "qhh": np.ascontiguousarray(np.asarray(inputs["q_Whh"], f32).T),
        "bq": chunk(np.asarray(inputs["q_bih"], f32)
                    + np.asarray(inputs["q_bhh"], f32)),
        "scoT": np.ascontiguousarray(np.asarray(inputs["score_w"], f32).T),
        "enhT": np.ascontiguousarray(np.asarray(inputs["enhance_w"], f32).T),
        "eb": chunk(inputs["enhance_b"]),
        "mskT": np.ascontiguousarray(np.asarray(inputs["mask_w"], f32).T),
        "mb": mb3,
        "mask_std": mask_std,
        "ident": np.eye(128, dtype=f32),
    }

    x_fm = np.ascontiguousarray(x.transpose(0, 2, 1))          # (8, 257, 2048)
    xpad = np.concatenate([np.zeros((B, FIN, 128), f32), x_fm], axis=2)
    in_maps = []
    for c in range(NCORES):
        m = dict(common)
        m["x_t"] = np.ascontiguousarray(xpad[:, :, c * OWN: c * OWN + WIN])
        m["mask_t0"] = mask_first if c == 0 else mask_std
        in_maps.append(m)
    return in_maps, x


def _host_post(results, x):
    f32 = np.float32
    y_full = np.empty((B, T, FIN), f32)
    w_full = np.zeros((B, T, T), f32)
    for c in range(NCORES):
        y_t = results[c]["y_t"]                       # (8, 257, 256)
        wb = results[c]["w_band"]                     # (8, 2, 128, 256)
        y_full[:, c * OWN:(c + 1) * OWN, :] = y_t.transpose(0, 2, 1)
        for t in range(2):
            r0 = c * OWN + t * 128
            s0 = r0 - 128
            if s0 < 0:
                w_full[:, r0:r0 + 128, 0:128] = wb[:, t, :, 128:]
            else:
                w_full[:, r0:r0 + 128, s0:s0 + 256] = wb[:, t]
    return y_full, w_full


def kernel(**inputs):
    nc = _get_nc()
    in_maps, x = _host_prep(inputs)
    res = bass_utils.run_bass_kernel_spmd(nc, in_maps,
                                          core_ids=list(range(NCORES)))
    return _host_post(res.results, x)


def run_with_trace(inputs, **kw):
    """Used by test.py for profiling."""
    nc = _get_nc()
    in_maps, x = _host_prep(inputs)
    res = bass_utils.run_bass_kernel_spmd(nc, in_maps,
                                          core_ids=list(range(NCORES)),
                                          trace=True, **kw)
    return _host_post(res.results, x), res
